# revision 9
# baseline (speedup 1.0000x reference)
"""Trainium2 Bass kernel for location-sensitive additive attention.

Reference computation (per batch element b):
    q       = query @ Wq.T + bias                        # [R]
    loc     = conv1d(cum_weights, conv_w, pad=15)        # [C, T]
    loc_feat= einsum('ct,rc->tr', loc, Wloc)             # [T, R]
    energy  = tanh(q + memory_transform + loc_feat)      # [T, R]
    e       = energy @ w_energy                          # [T]
    attn    = softmax(e)                                 # [T]
    context = attn @ memory                              # [MD]

Key algebraic fusion: loc_feat[t, r] = sum_k M[r, k] * cw_pad[t + k] with
M = Wloc @ conv_w[:, 0, :]  ([R, 31]) -- the C=32 channel dim contracts away,
so the whole location branch is ONE [31, T] x [31, R] matmul against an
im2col view of the (zero-padded) cumulative weights.

Sharding: data-parallel over batch B=64 across 8 cores (8 per core), all
weights replicated.  The mask input is all-ones per the problem spec and is
ignored.

Layout on device (per batch element):
  - psum_e [R=128 part, T free] accumulates the conv matmul plus 8 PE
    transposes of the naturally-laid-out memory_transform tiles.
  - ACT applies tanh with per-partition bias qb[r] straight out of PSUM.
  - energies = w_energy.T @ e on PE -> [1, T]; softmax on one partition
    (DVE max, ACT exp with accumulated sum, DVE reciprocal+scale).
  - attn [1, T] is PE-transposed into [128, 8] so the context reduction
    runs as 8 accumulating [128t, 1] x [128t, 512d] matmuls.
  - float32r (single-pass fp32 PE mode) for the N=512 streaming matmuls.
"""

import numpy as np

B, T = 64, 1024
R, QD, MD = 128, 1024, 512
C, K = 32, 31
PAD = (K - 1) // 2
NCORES = 8
BPC = B // NCORES  # batch elements per core
TCH = T // 128  # 128-column chunks of T

_cache = {}
_last_in_maps = None


def _patch_split_waits():
    """This walrus build accepts only one sync wait per instruction.
    Post-process the serialized BIR: hoist all but one wait of any
    instruction onto single-wait NoOps inserted just before it on the
    same engine (engines execute their stream in order, so this is
    semantically identical)."""
    import json
    import concourse.bass as bass

    if getattr(bass.Bass.to_json_bytes, "_split_waits", False):
        return
    orig = bass.Bass.to_json_bytes

    def to_json_bytes(self):
        m = json.loads(orig(self))
        n = 0
        for fn in m["functions"]:
            for blk in fn["blocks"]:
                out = []
                for inst in blk["instructions"]:
                    si = inst.get("sync_info")
                    waits = (si or {}).get("on_wait") or []
                    if len(waits) > 1:
                        for w in waits[:-1]:
                            n += 1
                            out.append({
                                "debug": inst.get("debug", 0),
                                "engine": inst["engine"],
                                "ins": [],
                                "name": f"I-waitsplit-{n}",
                                "opcode": "NoOp",
                                "outs": [],
                                "sync_info": {"on_update": [],
                                              "on_wait": [w]},
                            })
                        si["on_wait"] = [waits[-1]]
                    out.append(inst)
                blk["instructions"] = out
        return json.dumps(m).encode()

    to_json_bytes._split_waits = True
    bass.Bass.to_json_bytes = to_json_bytes


def _build():
    import concourse.bass as bass
    import concourse.mybir as mybir
    import concourse.tile as tile
    from concourse.masks import make_identity

    _patch_split_waits()

    f32 = mybir.dt.float32
    f32r = mybir.dt.float32r
    AF = mybir.ActivationFunctionType
    AX = mybir.AxisListType

    nc = bass.Bass(target_bir_lowering=False)

    q_in = nc.dram_tensor("q_in", [BPC, QD], f32, kind="ExternalInput")
    mem_in = nc.dram_tensor("mem_in", [BPC, T, MD], f32r, kind="ExternalInput")
    mt_in = nc.dram_tensor("mt_in", [BPC, T, R], f32, kind="ExternalInput")
    cw_in = nc.dram_tensor("cw_in", [BPC, T], f32r, kind="ExternalInput")
    wq_in = nc.dram_tensor("wq_in", [R, QD], f32, kind="ExternalInput")
    wloc_in = nc.dram_tensor("wloc_in", [R, C], f32, kind="ExternalInput")
    convw_in = nc.dram_tensor("convw_in", [C, K], f32, kind="ExternalInput")
    bias_in = nc.dram_tensor("bias_in", [1, R], f32, kind="ExternalInput")
    we_in = nc.dram_tensor("we_in", [1, R], f32, kind="ExternalInput")

    ctx_out = nc.dram_tensor("ctx_out", [BPC, MD], f32, kind="ExternalOutput")
    attn_out = nc.dram_tensor("attn_out", [BPC, T], f32, kind="ExternalOutput")

    TP = T + K - 1  # padded length 1054

    def r_(ap):
        return ap.bitcast(f32r)

    with tile.TileContext(nc) as tc:
        with tc.tile_pool(name="singles", bufs=1) as singles, \
             tc.tile_pool(name="dram", bufs=1, space="DRAM") as dram:

            identity = singles.tile([128, 128], f32)
            make_identity(nc, identity)
            ones11 = singles.tile([1, 1], f32)
            nc.vector.memset(ones11, 1.0)
            zeros16 = singles.tile([BPC, 16], f32)
            nc.vector.memset(zeros16, 0.0)

            wqT = singles.tile([128, QD], f32)  # col k*128+r = Wq[r, 128k+d']
            qT = singles.tile([128, QD // 128 * BPC], f32)  # col k*8+b
            qb = singles.tile([128, BPC], f32)  # q @ Wq.T + bias, col b
            bias_col = singles.tile([128, 1], f32)
            we_col = singles.tile([128, 1], f32r)
            mT = singles.tile([K, 128], f32r)  # fused conv+loc weight
            x_all = singles.tile([K, BPC, T], f32r)  # im2col of padded cw

            # ---- setup: transposes of the small operands -------------------
            with tc.tile_pool(name="setup_sb", bufs=1) as setup_sb, \
                 tc.tile_pool(name="setup_ps", bufs=2, space="PSUM") as setup_ps:
                wq_sb = setup_sb.tile([128, QD], f32)
                nc.sync.dma_start(out=wq_sb, in_=wq_in[:, :])
                q_sb = setup_sb.tile([BPC, QD], f32)
                nc.sync.dma_start(out=q_sb, in_=q_in[:, :])
                wloc_sb = setup_sb.tile([128, C], f32)
                nc.sync.dma_start(out=wloc_sb, in_=wloc_in[:, :])
                convw_sb = setup_sb.tile([C, K], f32)
                nc.sync.dma_start(out=convw_sb, in_=convw_in[:, :])
                bias_sb = setup_sb.tile([1, R], f32)
                nc.sync.dma_start(out=bias_sb, in_=bias_in[:, :])
                we_sb = setup_sb.tile([1, R], f32)
                nc.sync.dma_start(out=we_sb, in_=we_in[:, :])

                for k in range(QD // 128):
                    ps = setup_ps.tile([128, 128], f32, tag="sps")
                    nc.tensor.transpose(ps, wq_sb[:, k * 128:(k + 1) * 128],
                                        identity)
                    nc.vector.tensor_copy(wqT[:, k * 128:(k + 1) * 128], ps)
                    psq = setup_ps.tile([128, BPC], f32, tag="sps")
                    nc.tensor.transpose(psq, q_sb[:, k * 128:(k + 1) * 128],
                                        identity[:BPC, :BPC])
                    nc.vector.tensor_copy(qT[:, k * BPC:(k + 1) * BPC], psq)

                psb = setup_ps.tile([128, 1], f32, tag="sps")
                nc.tensor.transpose(psb, bias_sb, ones11)
                nc.vector.tensor_copy(bias_col, psb)
                psw = setup_ps.tile([128, 1], f32, tag="sps")
                nc.tensor.transpose(psw, we_sb, ones11)
                nc.vector.tensor_copy(we_col, psw)

                psl = setup_ps.tile([C, 128], f32, tag="sps")
                nc.tensor.transpose(psl, wloc_sb, identity)
                wlocT = setup_sb.tile([C, 128], f32)
                nc.vector.tensor_copy(wlocT, psl)

                # mT[k, r] = sum_c convw[c, k] * wlocT[c, r]
                psm = setup_ps.tile([K, 128], f32, tag="sps")
                nc.tensor.matmul(psm, lhsT=convw_sb, rhs=wlocT)
                nc.vector.tensor_copy(mT, psm)

                # qb[r, b] = sum_d Wq[r, d] query[b, d]  (+ bias[r])
                psqb = setup_ps.tile([128, BPC], f32, tag="sps")
                for k in range(QD // 128):
                    nc.tensor.matmul(
                        psqb,
                        lhsT=wqT[:, k * 128:(k + 1) * 128],
                        rhs=qT[:, k * BPC:(k + 1) * BPC],
                        start=(k == 0), stop=(k == QD // 128 - 1))
                nc.vector.tensor_scalar_add(qb, psqb, bias_col)

                # zero-padded cum_weights in DRAM, then one im2col DMA
                cw_sb = setup_sb.tile([BPC, T], f32r)
                nc.sync.dma_start(out=cw_sb, in_=cw_in[:, :])
                cwpad = dram.tile([BPC, TP], f32r)
                nc.sync.dma_start(out=cwpad[:, 0:PAD], in_=r_(zeros16[:, 0:PAD]))
                nc.sync.dma_start(out=cwpad[:, PAD:PAD + T], in_=cw_sb)
                nc.sync.dma_start(out=cwpad[:, PAD + T:TP],
                                  in_=r_(zeros16[:, 0:K - 1 - PAD]))
                x_src = bass.AP(tensor=cwpad.tensor, offset=cwpad.offset,
                                ap=[[1, K], [TP, BPC], [1, T]])
                nc.sync.dma_start(out=x_all, in_=x_src)

            # ---- steady state: one batch element at a time -----------------
            with tc.tile_pool(name="mem", bufs=2) as mem_pool, \
                 tc.tile_pool(name="mt", bufs=2) as mt_pool, \
                 tc.tile_pool(name="e", bufs=2) as e_pool, \
                 tc.tile_pool(name="sm", bufs=2) as sm_pool, \
                 tc.tile_pool(name="ps_e", bufs=2, space="PSUM") as ps_e_pool, \
                 tc.tile_pool(name="ps_en", bufs=1, space="PSUM") as ps_en_pool, \
                 tc.tile_pool(name="ps_sm", bufs=1, space="PSUM") as ps_sm_pool:
                for b in range(BPC):
                    mem_sb = mem_pool.tile([128, TCH, MD], f32r)
                    nc.sync.dma_start(
                        out=mem_sb,
                        in_=mem_in[b].rearrange("(c p) d -> p c d", p=128))
                    mt_sb = mt_pool.tile([128, TCH, R], f32)
                    nc.sync.dma_start(
                        out=mt_sb,
                        in_=mt_in[b].rearrange("(c p) r -> p c r", p=128))

                    # energy pre-activation: conv matmul + transposed mt
                    psum_e = ps_e_pool.tile([128, T], f32)
                    for h in range(2):
                        sl = slice(h * 512, (h + 1) * 512)
                        nc.tensor.matmul(
                            psum_e[:, sl],
                            lhsT=mT, rhs=x_all[:, b, sl],
                            start=True, stop=False, skip_group_check=True)
                        for j in range(4):
                            c = h * 4 + j
                            nc.tensor.matmul(
                                psum_e[:, c * 128:(c + 1) * 128],
                                lhsT=mt_sb[:, c, :], rhs=identity,
                                is_transpose=True, start=False, stop=(j == 3),
                                skip_group_check=True)

                    e_sb = e_pool.tile([128, T], f32r)
                    for h in range(2):
                        sl = slice(h * 512, (h + 1) * 512)
                        nc.scalar.activation(e_sb[:, sl], psum_e[:, sl],
                                             AF.Tanh, bias=qb[:, b:b + 1],
                                             scale=1.0)

                    # energies[t] = sum_r w_energy[r] * e[r, t]
                    psum_en = ps_en_pool.tile([1, T], f32)
                    for h in range(2):
                        sl = slice(h * 512, (h + 1) * 512)
                        nc.tensor.matmul(psum_en[0:1, sl], lhsT=we_col,
                                         rhs=e_sb[:, sl])

                    # softmax over T on one partition
                    negmax = sm_pool.tile([1, 1], f32, tag="negmax")
                    nc.vector.reduce_max(negmax, psum_en[0:1, :], axis=AX.X,
                                         negate=True)
                    attn_e = sm_pool.tile([1, T], f32, tag="attn_e")
                    esum = sm_pool.tile([1, 1], f32, tag="esum")
                    nc.scalar.activation(attn_e, psum_en[0:1, :], AF.Exp,
                                         bias=negmax, scale=1.0,
                                         accum_out=esum)
                    inv = sm_pool.tile([1, 1], f32, tag="inv")
                    nc.vector.reciprocal(inv, esum)
                    attn_sb = sm_pool.tile([1, T], f32, tag="attn")
                    nc.vector.tensor_scalar_mul(attn_sb, attn_e, inv)
                    nc.scalar.dma_start(out=attn_out[b:b + 1, :], in_=attn_sb)

                    # attn [1, T] -> [128, TCH] column layout for context
                    psum_at = ps_sm_pool.tile([128, TCH], f32, tag="at")
                    for c in range(TCH):
                        nc.tensor.matmul(
                            psum_at[:, c:c + 1],
                            lhsT=attn_sb[0:1, c * 128:(c + 1) * 128],
                            rhs=ones11, is_transpose=True)
                    attn_col = sm_pool.tile([128, TCH], f32r, tag="attn_col")
                    nc.vector.tensor_copy(attn_col, psum_at)

                    # context[d] = sum_t attn[t] * memory[t, d]
                    psum_ctx = ps_sm_pool.tile([1, MD], f32, tag="ctx")
                    for c in range(TCH):
                        nc.tensor.matmul(
                            psum_ctx,
                            lhsT=attn_col[:, c:c + 1],
                            rhs=mem_sb[:, c, :],
                            start=(c == 0), stop=(c == TCH - 1))
                    ctx_sb = sm_pool.tile([1, MD], f32, tag="ctx_sb")
                    nc.vector.tensor_copy(ctx_sb, psum_ctx)
                    nc.scalar.dma_start(out=ctx_out[b:b + 1, :], in_=ctx_sb)

    return nc


def kernel(query, memory, memory_transform, cum_weights, mask,
           Wq, Wloc, conv_w, bias, w_energy):
    from concourse.bass_utils import run_bass_kernel_spmd

    if "nc" not in _cache:
        _cache["nc"] = _build()
    nc = _cache["nc"]

    query = np.ascontiguousarray(np.asarray(query, dtype=np.float32))
    memory = np.ascontiguousarray(np.asarray(memory, dtype=np.float32))
    memory_transform = np.ascontiguousarray(
        np.asarray(memory_transform, dtype=np.float32))
    cum_weights = np.ascontiguousarray(np.asarray(cum_weights, dtype=np.float32))
    wq = np.ascontiguousarray(np.asarray(Wq, dtype=np.float32))
    wloc = np.ascontiguousarray(np.asarray(Wloc, dtype=np.float32))
    convw = np.ascontiguousarray(
        np.asarray(conv_w, dtype=np.float32).reshape(C, K))
    bias = np.ascontiguousarray(np.asarray(bias, dtype=np.float32))
    we = np.ascontiguousarray(np.asarray(w_energy, dtype=np.float32))

    in_maps = []
    for i in range(NCORES):
        s = slice(i * BPC, (i + 1) * BPC)
        in_maps.append({
            "q_in": np.ascontiguousarray(query[s]),
            "mem_in": np.ascontiguousarray(memory[s]),
            "mt_in": np.ascontiguousarray(memory_transform[s]),
            "cw_in": np.ascontiguousarray(cum_weights[s]),
            "wq_in": wq, "wloc_in": wloc, "convw_in": convw,
            "bias_in": bias, "we_in": we,
        })

    global _last_in_maps
    _last_in_maps = in_maps
    res = run_bass_kernel_spmd(nc, in_maps, core_ids=list(range(NCORES)))

    context = np.concatenate([r["ctx_out"] for r in res.results], axis=0)
    attn = np.concatenate([r["attn_out"] for r in res.results], axis=0)
    return context, attn[:, None, :]
